# revision 15
# baseline (speedup 1.0000x reference)
"""EnhancedBoundaryAttnPool Trainium2 kernel (v2).

Data-parallel over B=16 across 8 NeuronCores (2 batches/core).  Per batch:
  1. mean-pool init queries over boundary spans (span-union gathered, Tc=1408)
  2. boundary-masked cross attention (8 heads, d=128) over gathered positions
  3. add+LN, causal self-attention over 128 slots, add+LN.

v2 changes vs baseline:
  - all weights + matmul activations bf16 (fp32 only in PSUM/LN/residual path)
  - stage-major emission: each weight DMA'd once, used by both batches
    back-to-back; batch 0's vector-heavy stages overlap batch 1's matmuls
  - pgn/wtg shipped bf16 (DMA 92MB -> ~32MB per core)
  - cross-attn output accumulated in PSUM across t-tiles (no vector adds)
  - softmax: fused mask-mul + row-sum via tensor_tensor_reduce
"""
import math

import numpy as np
import ml_dtypes

import concourse.bass as bass
import concourse.tile as tile
from concourse import mybir
from concourse.bass_utils import run_bass_kernel_spmd

BF16 = ml_dtypes.bfloat16

B, T, K, H, NH = 16, 2048, 128, 1024, 8
D = H // NH                     # 128 head dim
NCORES = 8
BPC = B // NCORES               # batches per core
TC = 1408                       # padded span-union length (max observed 1356)
NTT = TC // 128                 # 11 t-tiles
CHUNKS = [(0, 512), (512, 512), (1024, 384)]
NHT = H // 128                  # 8 h-tiles
INV_SQRT_D = 1.0 / math.sqrt(D)

F32 = mybir.dt.float32
BF = mybir.dt.bfloat16


def split_multi_waits(nc):
    """walrus on this image rejects >1 sem-wait per instruction; move extras
    onto NoOps inserted just before, same engine."""
    n = 0
    for f in nc.m.functions:
        for blk in f.blocks:
            new_list = []
            for inst in blk.instructions:
                si = inst.sync_info
                if si is not None and len(si.on_wait) > 1:
                    waits = list(si.on_wait)
                    for k_, w in enumerate(waits[:-1]):
                        nop = mybir.InstNoOp(name=f"{inst.name}-wsplit{k_}",
                                             ins=[], outs=[])
                        nop.engine = inst.engine
                        nop.sync_info = mybir.SyncInfo(on_wait=[w], on_update=[])
                        new_list.append(nop)
                        n += 1
                    si.on_wait = [waits[-1]]
                new_list.append(inst)
            blk.instructions[:] = new_list
    return n


# ---------------------------------------------------------------- program ---

def _ln_apply(nc, pool, x_s, g_bc, b_bc, out_s, eps_t):
    """LayerNorm along free dim (1024) of x_s [128,1024] f32 -> out_s."""
    stats = pool.tile([128, 2, 6], F32, tag="ln_stats")
    mv = pool.tile([128, 2], F32, tag="ln_mv")
    for i in range(2):
        nc.vector.bn_stats(out=stats[:, i, :], in_=x_s[:, i * 512:(i + 1) * 512])
    nc.vector.bn_aggr(out=mv[:], in_=stats[:])
    rstd = pool.tile([128, 1], F32, tag="ln_rstd")
    nc.scalar.activation(out=rstd[:], in_=mv[:, 1:2],
                         func=mybir.ActivationFunctionType.Sqrt,
                         bias=eps_t[:], scale=1.0)
    nc.vector.reciprocal(out=rstd[:], in_=rstd[:])
    nc.vector.tensor_scalar(out=x_s[:], in0=x_s[:], scalar1=mv[:, 0:1],
                            scalar2=rstd[:], op0=mybir.AluOpType.subtract,
                            op1=mybir.AluOpType.mult)
    nc.vector.tensor_mul(out=x_s[:], in0=x_s[:], in1=g_bc[:])
    nc.vector.tensor_add(out=out_s[:], in0=x_s[:], in1=b_bc[:])


def build_program(for_sim=False):
    nc = bass.Bass()

    # --- DRAM I/O ---
    pgt_d = nc.dram_tensor("pgt", [BPC, NHT, 128, TC], BF, kind="ExternalInput")
    wtg_d = nc.dram_tensor("wtg", [BPC, NTT, 128, K], BF, kind="ExternalInput")
    mask_d = nc.dram_tensor("mask", [BPC, K, TC], BF, kind="ExternalInput")
    msa_d = nc.dram_tensor("msa", [BPC, K, K], BF, kind="ExternalInput")
    wnames = ["w_qp", "w_caq", "w_cak", "w_cav", "w_cao",
              "w_saq", "w_sak", "w_sav", "w_sao"]
    w_d = {n: nc.dram_tensor(n, [NHT, 128, H], BF, kind="ExternalInput")
           for n in wnames}
    # rows: 0 qp_b, 1 ca_bq, 2 ca_out_b, 3 sa_bq, 4 sa_bk, 5 sa_bv, 6 sa_out_b
    vrows_d = nc.dram_tensor("vrows", [7, H], BF, kind="ExternalInput")
    # cols: [128, 16]: 0:8 ca_bk (j-tiled), 8:16 ca_bv (j-tiled)
    vcols_d = nc.dram_tensor("vcols", [128, 16], F32, kind="ExternalInput")
    # LN vectors: 0 cn_g, 1 cn_b, 2 on_g, 3 on_b
    lng_d = nc.dram_tensor("lng", [4, H], BF, kind="ExternalInput")
    identb_d = nc.dram_tensor("identb", [128, 128], BF, kind="ExternalInput")
    ones_d = nc.dram_tensor("ones", [1, 128], BF, kind="ExternalInput")
    out_d = nc.dram_tensor("out", [BPC, K, H], F32, kind="ExternalOutput")

    with tile.TileContext(nc) as tc:
        with tc.tile_pool(name="const", bufs=1) as constp, \
             tc.tile_pool(name="wpool", bufs=4) as wpool, \
             tc.tile_pool(name="big", bufs=1) as bigp, \
             tc.tile_pool(name="acts", bufs=2) as actp, \
             tc.tile_pool(name="shared", bufs=2) as shp, \
             tc.tile_pool(name="trans", bufs=2) as trp, \
             tc.tile_pool(name="ps_sc", bufs=2, space="PSUM") as psp, \
             tc.tile_pool(name="ps_acc", bufs=2, space="PSUM") as psaccp, \
             tc.tile_pool(name="ps_tr", bufs=2, space="PSUM") as pstrp, \
             tc.tile_pool(name="ps_o", bufs=1, space="PSUM") as psop:

            # ---- constants (loaded once) ----
            ident_b = constp.tile([128, 128], BF)
            nc.sync.dma_start(ident_b[:], identb_d[:])
            ones_b = constp.tile([1, 128], BF)
            nc.sync.dma_start(ones_b[:], ones_d[:])
            vcols_s = constp.tile([128, 16], F32)
            nc.sync.dma_start(vcols_s[:], vcols_d[:])
            eps_t = constp.tile([128, 1], F32)
            nc.vector.memset(eps_t[:], 1e-5)

            vrow_cache = {}

            def vrow(r):
                if r not in vrow_cache:
                    t = trp.tile([1, H], BF, tag="vrow", bufs=2,
                                 name=f"vrow{r}")
                    nc.gpsimd.dma_start(t[:], vrows_d[r].unsqueeze(0))
                    vrow_cache[r] = t
                return vrow_cache[r]

            lnbc_t = {}

            def ln_bc(row):
                if row not in lnbc_t:
                    t = constp.tile([128, H], BF, name=f"lnbc{row}")
                    src = lng_d[row]
                    bcast = bass.AP(tensor=src.tensor, offset=src.offset,
                                    ap=[[0, 128]] + [list(p) for p in src.ap])
                    nc.sync.dma_start(t[:], bcast)
                    lnbc_t[row] = t
                return lnbc_t[row]

            class WPair:
                def __init__(self, halves):
                    self.h = halves

                def __getitem__(self, idx):
                    p, ht, js = idx
                    return self.h[ht // 4][p, ht % 4, js]

            def wload(name):
                halves = []
                for hf in range(2):
                    t = wpool.tile([128, 4, H], BF, tag="w",
                                   name=f"w_{name}_{hf}")
                    nc.sync.dma_start(
                        t[:],
                        w_d[name][hf * 4:(hf + 1) * 4].rearrange(
                            "nh p j -> p nh j"))
                    halves.append(t)
                return WPair(halves)

            def mm_chunks(out_psums, lhsT_tiles, rhs_of, bias_row=None,
                          chunk_sizes=((0, 512), (512, 512))):
                """acc over NHT h-tiles into psum chunks; optional bias row."""
                brow_t = vrow(bias_row) if bias_row is not None else None
                for ci, (off, sz) in enumerate(chunk_sizes):
                    for ht in range(NHT):
                        nc.tensor.matmul(
                            out_psums[ci][:, :sz], lhsT_tiles(ht),
                            rhs_of(ht, off, sz),
                            start=(ht == 0),
                            stop=(ht == NHT - 1 and bias_row is None))
                    if bias_row is not None:
                        nc.tensor.matmul(
                            out_psums[ci][:, :sz], ones_b[:],
                            brow_t[:, off:off + sz],
                            start=False, stop=True)

            def transpose8(src_s, out_tag, pool=actp, bufs=None):
                """transpose [128, 1024] bf16 (8 col blocks) -> [128, 8, 128]."""
                dst = pool.tile([128, NHT, 128], BF, tag=out_tag, bufs=bufs)
                for ht in range(NHT):
                    ps = pstrp.tile([128, 128], BF, tag="tr")
                    nc.tensor.transpose(
                        ps[:], src_s[:, ht * 128:(ht + 1) * 128], ident_b[:])
                    if ht % 2 == 0:
                        nc.vector.tensor_copy(dst[:, ht, :], ps[:])
                    else:
                        nc.scalar.copy(dst[:, ht, :], ps[:])
                return dst

            S = [dict() for _ in range(BPC)]

            # ====== stage 1: mean-pool init slots (pg^T transposed on PE) ====
            # DMA order: pgT b0, wtg b0, w_qp, pgT b1, ... so PE has dense
            # transpose+matmul work from ~9us in and HAM warms early.
            w_qp_s = None
            for b in range(BPC):
                pgT = bigp.tile([128, NHT, TC], BF, tag="pgT", bufs=2)
                for ht in range(NHT):
                    nc.sync.dma_start(pgT[:, ht, :], pgt_d[b, ht])
                wtg_tiles = []
                for tt in range(NTT):
                    wtg_t = trp.tile([128, K], BF, tag="wtg", bufs=4,
                                     name=f"wtg{b}_{tt}")
                    nc.sync.dma_start(wtg_t[:], wtg_d[b, tt])
                    wtg_tiles.append(wtg_t)
                S[b]["pgT"] = pgT
                if b == 0:
                    w_qp_s = wload("w_qp")
                init_ps = [psaccp.tile([128, 512], F32, tag="acc",
                                       name=f"initps{b}_{i_}")
                           for i_ in range(2)]
                for tt in range(NTT):
                    pgnT_t = trp.tile([128, NHT, 128], BF, tag="pgnT", bufs=2)
                    for ht in range(NHT):
                        tps = pstrp.tile([128, 128], BF, tag="tr")
                        nc.tensor.transpose(
                            tps[:], pgT[:, ht, tt * 128:(tt + 1) * 128],
                            ident_b[:])
                        if ht % 2 == 0:
                            nc.vector.tensor_copy(pgnT_t[:, ht, :], tps[:])
                        else:
                            nc.scalar.copy(pgnT_t[:, ht, :], tps[:])
                    for ci in range(2):
                        nc.tensor.matmul(init_ps[ci][:], wtg_tiles[tt][:],
                                         pgnT_t[:, 4 * ci:4 * ci + 4, :],
                                         start=(tt == 0), stop=(tt == NTT - 1))
                init_bf = shp.tile([128, H], BF, tag="init_bf")
                for ci in range(2):
                    nc.vector.tensor_copy(init_bf[:, ci * 512:(ci + 1) * 512],
                                          init_ps[ci][:])
                S[b]["init_bf"] = init_bf

            # mask prefetch (needed from stage 4A)
            for b in range(BPC):
                mask_s = bigp.tile([128, TC], BF, tag="mask", bufs=2)
                nc.gpsimd.dma_start(mask_s[:], mask_d[b])
                S[b]["mask"] = mask_s

            # ============ stage 2: queries = init @ qp_w.T + qp_b ============
            for b in range(BPC):
                initT = transpose8(S[b]["init_bf"], "t8a", pool=shp)
                q_ps = [psaccp.tile([128, 512], F32, tag="acc",
                                    name=f"qps{b}_{i_}") for i_ in range(2)]
                mm_chunks(q_ps, lambda ht: initT[:, ht, :],
                          lambda ht, off, sz: w_qp_s[:, ht, off:off + sz],
                          bias_row=0)
                queries_bf = actp.tile([128, H], BF, tag="queries")
                for ci in range(2):
                    nc.scalar.copy(queries_bf[:, ci * 512:(ci + 1) * 512],
                                   q_ps[ci][:])
                S[b]["queries"] = queries_bf

            # ======== stage 3: qh = queries @ wq.T + bq -> qhT bf16 ==========
            w_caq_s = wload("w_caq")
            for b in range(BPC):
                queriesT = transpose8(S[b]["queries"], "t8a", pool=shp)
                qh_ps = [psaccp.tile([128, 512], F32, tag="acc",
                                     name=f"qhps{b}_{i_}") for i_ in range(2)]
                mm_chunks(qh_ps, lambda ht: queriesT[:, ht, :],
                          lambda ht, off, sz: w_caq_s[:, ht, off:off + sz],
                          bias_row=1)
                qh_s = shp.tile([128, H], BF, tag="xb")
                for ci in range(2):
                    nc.vector.tensor_copy(qh_s[:, ci * 512:(ci + 1) * 512],
                                          qh_ps[ci][:])
                S[b]["qhT"] = transpose8(qh_s, "qhT")

            # ============== stage 4A: kh + scores + masked exp ===============
            w_cak_s = wload("w_cak")
            for b in range(BPC):
                pgT, mask_s = S[b]["pgT"], S[b]["mask"]
                attn_s = bigp.tile([128, NH, TC], BF, tag="attn", bufs=2,
                                   name=f"attn{b}")
                lsums = actp.tile([128, NH * 4], F32, tag="lsums")
                qhT = S[b]["qhT"]
                for c_i, (off, sz) in enumerate(CHUNKS):
                    for jt in range(NHT):
                        kps = psaccp.tile([128, 512], F32, tag="acc",
                                          name=f"kps{b}_{off}_{jt}")
                        for ht in range(NHT):
                            nc.tensor.matmul(
                                kps[:, :sz],
                                w_cak_s[:, ht, jt * 128:(jt + 1) * 128],
                                pgT[:, ht, off:off + sz],
                                start=(ht == 0), stop=(ht == NHT - 1))
                        khT_blk = trp.tile([128, 512], BF, tag="khT", bufs=2)
                        # ca_bk adds a per-(slot,head) constant to scores;
                        # it cancels in softmax -> plain copy
                        if jt % 2 == 0:
                            nc.vector.tensor_copy(khT_blk[:, :sz],
                                                  kps[:, :sz])
                        else:
                            nc.scalar.copy(khT_blk[:, :sz], kps[:, :sz])
                        sps = psp.tile([128, 512], F32, tag="sps")
                        nc.tensor.matmul(sps[:, :sz], qhT[:, jt, :],
                                         khT_blk[:, :sz], start=True, stop=True)
                        nc.scalar.activation(
                            attn_s[:, jt, off:off + sz], sps[:, :sz],
                            func=mybir.ActivationFunctionType.Exp,
                            scale=INV_SQRT_D)
                        # fused mask mul + partial row-sum
                        nc.vector.scalar_tensor_tensor(
                            out=attn_s[:, jt, off:off + sz],
                            in0=attn_s[:, jt, off:off + sz],
                            scalar=1.0,
                            in1=mask_s[:, off:off + sz],
                            op0=mybir.AluOpType.mult,
                            op1=mybir.AluOpType.mult,
                            accum_out=lsums[:, jt * 4 + c_i:jt * 4 + c_i + 1])
                # total sums + recip + normalize
                rec = actp.tile([128, NH], F32, tag="rec")
                lsums3 = bass.AP(tensor=lsums.tensor, offset=lsums.offset,
                                 ap=[list(lsums.ap[0]), [4, NH], [1, 3]])
                nc.vector.reduce_sum(rec[:], lsums3, axis=mybir.AxisListType.X)
                nc.vector.reciprocal(rec[:], rec[:])
                for h in range(NH):
                    nc.vector.tensor_scalar_mul(attn_s[:, h, :],
                                                attn_s[:, h, :],
                                                rec[:, h:h + 1])
                S[b]["attn"] = attn_s

            # ========== stage 4B: vh per t-tile, o accumulate in PSUM ========
            w_cav_s = wload("w_cav")
            for b in range(BPC):
                pgT, attn_s = S[b]["pgT"], S[b]["attn"]
                oacc = psop.tile([128, NH, 128], F32, tag="oacc",
                                 name=f"oacc{b}")
                for tt in range(NTT):
                    vh_t = trp.tile([128, H], BF, tag="tmp1024")
                    for ci in range(2):
                        vps = psaccp.tile([128, 512], F32, tag="acc",
                                          name=f"vps{b}_{tt}_{ci}")
                        for ht in range(NHT):
                            nc.tensor.matmul(
                                vps[:], pgT[:, ht, tt * 128:(tt + 1) * 128],
                                w_cav_s[:, ht, ci * 512:(ci + 1) * 512],
                                start=(ht == 0), stop=(ht == NHT - 1))
                        nc.scalar.copy(vh_t[:, ci * 512:(ci + 1) * 512],
                                       vps[:])
                    for h in range(NH):
                        attnT_t = trp.tile([128, 128], BF, tag="attnT", bufs=4)
                        tps = pstrp.tile([128, 128], BF, tag="tr")
                        nc.tensor.transpose(
                            tps[:], attn_s[:, h, tt * 128:(tt + 1) * 128],
                            ident_b[:])
                        nc.vector.tensor_copy(attnT_t[:], tps[:])
                        nc.tensor.matmul(
                            oacc[:, h, :], vh_t[:, h * 128:(h + 1) * 128],
                            attnT_t[:],
                            start=(tt == 0 and h % 4 == 0),
                            stop=(tt == NTT - 1 and h % 4 == 3))
                acat_s = actp.tile([128, NHT, 128], BF, tag="acat")
                for h in range(NH):
                    nc.vector.tensor_scalar_add(
                        acat_s[:, h, :], oacc[:, h, :],
                        vcols_s[:, h:h + 1])
                S[b]["acat"] = acat_s

            # ============ stage 5: CA out proj + residual + LN ===============
            w_cao_s = wload("w_cao")
            for b in range(BPC):
                so_ps = [psaccp.tile([128, 512], F32, tag="acc",
                                     name=f"sops{b}_{i_}") for i_ in range(2)]
                mm_chunks(so_ps, lambda ht: S[b]["acat"][:, ht, :],
                          lambda ht, off, sz: w_cao_s[:, ht, off:off + sz],
                          bias_row=2)
                x_s = shp.tile([128, H], F32, tag="x_f32")
                for ci in range(2):
                    nc.vector.tensor_add(x_s[:, ci * 512:(ci + 1) * 512],
                                         so_ps[ci][:],
                                         S[b]["queries"][:, ci * 512:(ci + 1) * 512])
                slots_bf = actp.tile([128, H], BF, tag="slots")
                _ln_apply(nc, shp, x_s, ln_bc(0), ln_bc(1), slots_bf, eps_t)
                S[b]["slots"] = slots_bf
                msa_s = actp.tile([128, K], BF, tag="msa")
                nc.gpsimd.dma_start(msa_s[:], msa_d[b])
                S[b]["msa"] = msa_s

            # ============= stage 6: self-attention over slots ================
            # projection-major: each weight's uses (both batches) complete
            # before the next weight's slot is needed (avoids slot deadlock)
            for b in range(BPC):
                S[b]["slotsT"] = transpose8(S[b]["slots"], "t8b", pool=shp,
                                            bufs=6)
            for wname, brow, nm in [("w_saq", 3, "qsaT"),
                                    ("w_sak", None, "ksaT")]:
                w_s = wload(wname)
                for b in range(BPC):
                    pps = [psaccp.tile([128, 512], F32, tag="acc",
                                       name=f"pps{b}_{nm}_{i_}")
                           for i_ in range(2)]
                    mm_chunks(pps, lambda ht: S[b]["slotsT"][:, ht, :],
                              lambda ht, off, sz: w_s[:, ht, off:off + sz],
                              bias_row=brow)
                    xb = shp.tile([128, H], BF, tag="xb")
                    for ci in range(2):
                        nc.vector.tensor_copy(xb[:, ci * 512:(ci + 1) * 512],
                                              pps[ci][:])
                    S[b][nm] = transpose8(xb, "t8b", pool=shp, bufs=6)
            w_sav_s = wload("w_sav")
            for b in range(BPC):
                vps2 = [psaccp.tile([128, 512], F32, tag="acc",
                                    name=f"vps2{b}_{i_}") for i_ in range(2)]
                mm_chunks(vps2, lambda ht: S[b]["slotsT"][:, ht, :],
                          lambda ht, off, sz: w_sav_s[:, ht, off:off + sz])
                vhsa_s = actp.tile([128, H], BF, tag="vhsa")
                for ci in range(2):
                    nc.scalar.copy(vhsa_s[:, ci * 512:(ci + 1) * 512],
                                   vps2[ci][:])
                S[b]["vhsa"] = vhsa_s
            w_sao_s = wload("w_sao")
            for b in range(BPC):
                vhsa_s = S[b]["vhsa"]
                qkv_T = {"qsaT": S[b]["qsaT"], "ksaT": S[b]["ksaT"]}
                msa_s = S[b]["msa"]
                ocat_s = actp.tile([128, NHT, 128], BF, tag="ocat")
                lsum2 = shp.tile([128, 1], F32, tag="lsum")
                for h in range(NH):
                    scps = psp.tile([128, 128], F32, tag="sps",
                                    name=f"scps{b}_{h}")
                    nc.tensor.matmul(scps[:], qkv_T["qsaT"][:, h, :],
                                     qkv_T["ksaT"][:, h, :],
                                     start=True, stop=True)
                    asa = trp.tile([128, K], BF, tag="asa")
                    nc.scalar.activation(asa[:], scps[:],
                                         func=mybir.ActivationFunctionType.Exp,
                                         scale=INV_SQRT_D)
                    nc.vector.scalar_tensor_tensor(
                        out=asa[:], in0=asa[:], scalar=1.0, in1=msa_s[:],
                        op0=mybir.AluOpType.mult,
                        op1=mybir.AluOpType.mult,
                        accum_out=lsum2[:])
                    nc.vector.reciprocal(lsum2[:], lsum2[:])
                    nc.vector.tensor_scalar_mul(asa[:], asa[:], lsum2[:])
                    asaT = trp.tile([128, 128], BF, tag="attnT", bufs=4)
                    tps2 = pstrp.tile([128, 128], BF, tag="tr")
                    nc.tensor.transpose(tps2[:], asa[:], ident_b[:])
                    nc.vector.tensor_copy(asaT[:], tps2[:])
                    osps = psp.tile([128, 128], F32, tag="sps",
                                    name=f"osps{b}_{h}")
                    nc.tensor.matmul(osps[:],
                                     vhsa_s[:, h * 128:(h + 1) * 128], asaT[:],
                                     start=True, stop=True)
                    # sa_bv folds to +bv after softmax (rows sum to 1)
                    nc.vector.tensor_scalar_add(ocat_s[:, h, :], osps[:],
                                                vcols_s[:, 8 + h:9 + h])
                S[b]["ocat"] = ocat_s

                # ---- stage 7 for this batch (keeps PE fed through tail) ----
                ctx_ps = [psaccp.tile([128, 512], F32, tag="acc",
                                      name=f"ctxps{b}_{i_}")
                          for i_ in range(2)]
                mm_chunks(ctx_ps, lambda ht: ocat_s[:, ht, :],
                          lambda ht, off, sz: w_sao_s[:, ht, off:off + sz],
                          bias_row=6)
                x2_s = shp.tile([128, H], F32, tag="x_f32")
                for ci in range(2):
                    nc.vector.tensor_add(x2_s[:, ci * 512:(ci + 1) * 512],
                                         ctx_ps[ci][:],
                                         S[b]["slots"][:, ci * 512:(ci + 1) * 512])
                _ln_apply(nc, shp, x2_s, ln_bc(2), ln_bc(3), x2_s, eps_t)
                nc.sync.dma_start(out_d[b], x2_s[:])

    nc.finalize()
    if not for_sim:
        split_multi_waits(nc)
    return nc


# ------------------------------------------------------------- host side ---

def _prep_inputs(projected, boundaries, slot_mask, qp_w, qp_b, ca_in_w,
                 ca_in_b, ca_out_w, ca_out_b, cn_g, cn_b, sa_in_w, sa_in_b,
                 sa_out_w, sa_out_b, on_g, on_b):
    projected = np.asarray(projected, np.float32)
    boundaries = np.asarray(boundaries)
    slot_mask = np.asarray(slot_mask, np.float32)

    def wt(w):  # (H,H) -> transposed, tiled [NHT, 128, H], bf16
        return np.ascontiguousarray(
            np.asarray(w, np.float32).T.reshape(NHT, 128, H)).astype(BF16)

    ca_in_w = np.asarray(ca_in_w, np.float32)
    sa_in_w = np.asarray(sa_in_w, np.float32)
    weights = {
        "w_qp": wt(qp_w),
        "w_caq": wt(ca_in_w[:H]),
        "w_cak": wt(ca_in_w[H:2 * H]),
        "w_cav": wt(ca_in_w[2 * H:]), "w_cao": wt(ca_out_w),
        "w_saq": wt(sa_in_w[:H]), "w_sak": wt(sa_in_w[H:2 * H]),
        "w_sav": wt(sa_in_w[2 * H:]), "w_sao": wt(sa_out_w),
    }
    ca_in_b = np.asarray(ca_in_b, np.float32)
    sa_in_b = np.asarray(sa_in_b, np.float32)
    vrows = np.stack([
        np.asarray(qp_b, np.float32), ca_in_b[:H],
        np.asarray(ca_out_b, np.float32), sa_in_b[:H], sa_in_b[H:2 * H],
        sa_in_b[2 * H:], np.asarray(sa_out_b, np.float32)]).astype(BF16)
    vcols = np.concatenate([
        ca_in_b[2 * H:].reshape(NHT, 128).T,       # ca_bv
        sa_in_b[2 * H:].reshape(NHT, 128).T], 1)   # sa_bv
    vcols = np.ascontiguousarray(vcols, np.float32)
    lng = np.stack([np.asarray(v, np.float32)
                    for v in (cn_g, cn_b, on_g, on_b)]).astype(BF16)

    tidx = np.arange(T)
    starts = boundaries[:, :, 0].astype(np.int64)
    ends = boundaries[:, :, 1].astype(np.int64)

    per_core = []
    for c in range(NCORES):
        pgt = np.zeros((BPC, NHT, 128, TC), np.float32)
        wtg = np.zeros((BPC, NTT, 128, K), np.float32)
        maskg = np.zeros((BPC, K, TC), np.float32)
        msa = np.zeros((BPC, K, K), np.float32)
        for bi in range(BPC):
            i = c * BPC + bi
            in_bkt = (tidx[None, :] >= starts[i][:, None]) & \
                     (tidx[None, :] < ends[i][:, None])          # (K, T)
            valid = slot_mask[i] > 0.5
            in_slot = (in_bkt & (slot_mask[i][:, None] > 0)).astype(np.float32)
            w = in_slot / np.clip(in_slot.sum(-1, keepdims=True), 1.0, None)
            allowed = in_bkt & valid[:, None]                    # (K, T)
            t_idx = np.flatnonzero(allowed.any(0))
            ncov = len(t_idx)
            t_full = np.zeros(TC, np.int64)
            t_full[:ncov] = t_idx
            pgt[bi] = projected[i][t_full].T.reshape(NHT, 128, TC)
            wg = w[:, t_full].copy()
            wg[:, ncov:] = 0.0
            wtg[bi] = wg.T.reshape(NTT, 128, K)
            mg = allowed[:, t_full].astype(np.float32)
            mg[:, ncov:] = 0.0
            maskg[bi] = mg
            causal = np.tril(np.ones((K, K), np.float32))
            msa[bi] = causal * (slot_mask[i][None, :] > 0.5)
        per_core.append({
            "pgt": pgt.astype(BF16),
            "wtg": wtg.astype(BF16),
            "mask": maskg.astype(BF16), "msa": msa.astype(BF16),
            "vrows": vrows, "vcols": vcols, "lng": lng,
            "identb": np.eye(128, dtype=BF16),
            "ones": np.ones((1, 128), BF16), **weights})
    return per_core


_NC_CACHE = {}


def _get_nc():
    if "nc" not in _NC_CACHE:
        _NC_CACHE["nc"] = build_program()
    return _NC_CACHE["nc"]


def run_in_maps(in_maps, trace=False, **kw):
    nc = _get_nc()
    return run_bass_kernel_spmd(nc, in_maps, list(range(NCORES)),
                                trace=trace, **kw)


def kernel(**inputs) -> np.ndarray:
    in_maps = _prep_inputs(**inputs)
    res = run_in_maps(in_maps)
    out = np.zeros((B, K, H), np.float32)
    for c in range(NCORES):
        out[c * BPC:(c + 1) * BPC] = res.results[c]["out"]
    return out
